# revision 1
# baseline (speedup 1.0000x reference)
"""CAM (channel attention) kernel for Trainium2, SPMD over 8 NeuronCores.

Full inputs: x [16, 512, 64, 64] f32, gamma [1] f32.
Math per batch b (N = 64*64 = 4096 pixels, C = 512 channels):
    q = x[b].reshape(C, N)
    E = q @ q.T                            # (C, C)
    A = softmax(rowmax(E) - E, axis=-1)    # == exp(rowmin(E) - E) / rowsum
    y[b] = gamma * (A @ q) + x[b]

Sharding: data-parallel over batch. Each core takes 2 of the 16 batch
elements; no cross-core communication.

Precision: matmuls in bf16 with fp32 PSUM accumulation (validated vs the
fp32 reference: L2 rel err ~6e-4, maxabs/refmax ~6e-3); softmax pipeline
and the final residual add in fp32.

Transposes (q^T for the Gram matmul, A^T for the second matmul) run on
the TensorEngine via identity matmuls staged through PSUM. Batch 1's
load/cast/transpose/E phase is interleaved into batch 0's output phase
so the TensorEngine never idles between batches.
"""

from contextlib import ExitStack

import numpy as np

import concourse.bacc as bacc
import concourse.bass as bass
import concourse.mybir as mybir
import concourse.tile as tile
from concourse.bass_utils import run_bass_kernel_spmd
from concourse.masks import make_identity

P = 128            # SBUF partitions
C = 512            # channels
CT = C // P        # 4 channel chunks
NPIX = 4096        # H*W
SL = 512           # pixel-slice width
NS = NPIX // SL    # 8 pixel slices
KT = NPIX // P     # 32 contraction chunks for E
MB = 2             # batch elements per core
NCORES = 8

F32 = mybir.dt.float32
BF16 = mybir.dt.bfloat16
AX = mybir.AxisListType.X
MIN = mybir.AluOpType.min
EXP = mybir.ActivationFunctionType.Exp
COPY = mybir.ActivationFunctionType.Copy


def build_nc() -> bacc.Bacc:
    nc = bacc.Bacc("TRN2", target_bir_lowering=False, debug=False)
    x = nc.declare_dram_parameter("x", [MB, C, 64, 64], F32, isOutput=False)
    g = nc.declare_dram_parameter("gamma", [1], F32, isOutput=False)
    y = nc.declare_dram_parameter("y", [MB, C, 64, 64], F32, isOutput=True)

    xv = x[:].rearrange("b (t p) h w -> b t p (h w)", p=P)
    yv = y[:].rearrange("b (t p) h w -> b t p (h w)", p=P)

    with tile.TileContext(nc) as tc, ExitStack() as ctx:
        xpool = ctx.enter_context(tc.tile_pool(name="x", bufs=36))
        qhpool = ctx.enter_context(tc.tile_pool(name="qh", bufs=38))
        qhtpool = ctx.enter_context(tc.tile_pool(name="qht", bufs=2))
        upool = ctx.enter_context(tc.tile_pool(name="u", bufs=3))
        apool = ctx.enter_context(tc.tile_pool(name="a", bufs=8))
        atpool = ctx.enter_context(tc.tile_pool(name="at", bufs=2))
        ypool = ctx.enter_context(tc.tile_pool(name="y", bufs=4))
        stat = ctx.enter_context(tc.tile_pool(name="stat", bufs=16))
        cpool = ctx.enter_context(tc.tile_pool(name="const", bufs=1))
        epsum = ctx.enter_context(tc.tile_pool(name="epsum", bufs=1, space="PSUM"))
        tpsum = ctx.enter_context(tc.tile_pool(name="tpsum", bufs=2, space="PSUM"))
        opsum = ctx.enter_context(tc.tile_pool(name="opsum", bufs=2, space="PSUM"))

        gamma_b = cpool.tile([P, 1], F32)
        nc.gpsimd.dma_start(gamma_b[:], g[:].to_broadcast((P, 1)))
        ident = cpool.tile([P, P], BF16)
        make_identity(nc, ident[:])

        # per-batch state
        st = [dict(x_t={}, qh_t={}, qht=None, e_ps=None, at_t=None)
              for _ in range(MB)]

        def a_chunk(b, ns):
            """Load pixel-slice ns of batch b, cast, transpose into qht."""
            s = st[b]
            for ct in range(CT):
                xt = xpool.tile([P, SL], F32, tag="x", name="xt")
                nc.gpsimd.dma_start(xt[:], xv[b, ct, :, ns * SL:(ns + 1) * SL])
                s["x_t"][ct, ns] = xt
                qt = qhpool.tile([P, SL], BF16, tag="qh", name="qt")
                nc.scalar.copy(qt[:], xt[:])
                s["qh_t"][ct, ns] = qt
                # PE transpose: tp[p, kk*P + c'] = qt[c', kk*P + p]
                tp = tpsum.tile([P, SL], BF16, tag="tp", name="tp")
                for kk in range(4):
                    nc.tensor.transpose(
                        tp[:, kk * P:(kk + 1) * P],
                        qt[:, kk * P:(kk + 1) * P],
                        ident[:],
                    )
                # copy PSUM -> qht[:, 4ns..4ns+4, ct-chunk]
                dst = s["qht"][:, 4 * ns:4 * ns + 4, ct * P:(ct + 1) * P]
                if ct % 2 == 0:
                    nc.vector.tensor_copy(dst, tp[:])
                else:
                    nc.scalar.copy(dst, tp[:])

        def e_mms(b, ns):
            """E-accumulation matmuls for pixel-slice ns of batch b."""
            s = st[b]
            qht = s["qht"]
            for m in range(CT):
                for kk in range(4):
                    k = 4 * ns + kk
                    nc.tensor.matmul(
                        s["e_ps"][m][:, :],
                        qht[:, k, m * P:(m + 1) * P],
                        qht[:, k, :],
                        start=(k == 0),
                        stop=(k == KT - 1),
                    )

        def softmax(b):
            """A = gamma * exp(min - E) / rowsum; build A^T via PE."""
            s = st[b]
            at_t = atpool.tile([P, CT, C], BF16, tag="at", name="at_t")
            s["at_t"] = at_t
            for m in range(CT):
                e = s["e_ps"][m]
                mn = stat.tile([P, 1], F32, tag="mn", name="mn")
                nc.vector.tensor_reduce(mn[:], e[:], AX, MIN)
                u = upool.tile([P, C], F32, tag="u", name="u")
                sm = stat.tile([P, 1], F32, tag="sm", name="sm")
                nc.scalar.activation(
                    u[:], e[:], EXP, bias=mn[:], scale=-1.0, accum_out=sm[:]
                )
                rc = stat.tile([P, 1], F32, tag="rc", name="rc")
                nc.vector.reciprocal(rc[:], sm[:])
                sc = stat.tile([P, 1], F32, tag="sc", name="sc")
                nc.vector.tensor_scalar_mul(sc[:], rc[:], gamma_b[:])
                a = apool.tile([P, C], BF16, tag="a", name="a")
                nc.vector.tensor_scalar_mul(a[:], u[:], sc[:])
                tp2 = tpsum.tile([P, SL], BF16, tag="tp", name="tp2")
                for kk in range(4):
                    nc.tensor.transpose(
                        tp2[:, kk * P:(kk + 1) * P],
                        a[:, kk * P:(kk + 1) * P],
                        ident[:],
                    )
                nc.vector.tensor_copy(at_t[:, :, m * P:(m + 1) * P], tp2[:])

        def b_chunk(b, ns):
            """out = A @ qh for pixel-slice ns; add residual; store."""
            s = st[b]
            for m in range(CT):
                ops = opsum.tile([P, SL], F32, tag="o", name="ops")
                for k in range(CT):
                    nc.tensor.matmul(
                        ops[:],
                        s["at_t"][:, k, m * P:(m + 1) * P],
                        s["qh_t"][k, ns][:],
                        start=(k == 0),
                        stop=(k == CT - 1),
                    )
                yt = ypool.tile([P, SL], F32, tag="y", name="yt")
                nc.vector.tensor_add(yt[:], ops[:], s["x_t"][m, ns][:])
                nc.gpsimd.dma_start(yv[b, m, :, ns * SL:(ns + 1) * SL], yt[:])

        def alloc_batch(b):
            s = st[b]
            # qht[p, k, c] = qh[c, k*P + p]   (q^T, bf16)
            s["qht"] = qhtpool.tile([P, KT, C], BF16, tag="qht", name="qht")
            s["e_ps"] = [
                epsum.tile([P, C], F32, tag=f"e{m}", name=f"e_ps{m}")
                for m in range(CT)
            ]

        # ---- batch 0 phase A (with one-slice E lag) ----
        alloc_batch(0)
        for ns in range(NS):
            a_chunk(0, ns)
            if ns > 0:
                e_mms(0, ns - 1)
        e_mms(0, NS - 1)

        # batch 1's first slice keeps PE busy during batch 0's softmax
        alloc_batch(1)
        a_chunk(1, 0)
        softmax(0)

        # ---- interleave: batch 0 output phase + batch 1 input phase ----
        for j in range(NS):
            b_chunk(0, j)
            if j + 1 < NS:
                a_chunk(1, j + 1)
            if j > 0:
                e_mms(1, j - 1)
        e_mms(1, NS - 1)

        softmax(1)
        for ns in range(NS):
            b_chunk(1, ns)

    return nc


_NC = None


def _get_nc() -> bacc.Bacc:
    global _NC
    if _NC is None:
        _NC = build_nc()
        _NC.finalize()
    return _NC


def _run(x: np.ndarray, gamma: np.ndarray, trace: bool = False):
    x = np.ascontiguousarray(x, dtype=np.float32)
    gamma = np.ascontiguousarray(gamma, dtype=np.float32).reshape(1)
    in_maps = [
        {"x": x[MB * i:MB * (i + 1)], "gamma": gamma} for i in range(NCORES)
    ]
    res = run_bass_kernel_spmd(
        _get_nc(), in_maps, core_ids=list(range(NCORES)), trace=trace
    )
    out = np.concatenate([r["y"] for r in res.results], axis=0)
    return out.astype(np.float32, copy=False), res


def kernel(x: np.ndarray, gamma: np.ndarray) -> np.ndarray:
    out, _ = _run(x, gamma, trace=False)
    return out


def kernel_profiled(x: np.ndarray, gamma: np.ndarray):
    out, res = _run(x, gamma, trace=True)
    return out, res

